# revision 25
# baseline (speedup 1.0000x reference)
"""AtomicaDynamics EGNN/cross-attention message passing on 8 TRN2 NeuronCores.

Data-parallel over graphs (4 graphs/core, edges never cross graphs).
Dense all-pairs formulation: per graph, ll messages over the complete 64x64
grid (diagonal corrected), lp messages over the complete 64x128 bipartite
grid. Two graphs are packed onto the 128 SBUF partitions (feature-major pair
tensors), with block-diagonal weights.
"""
import sys

sys.path.insert(0, "/opt/trn_rl_repo")

import os
import numpy as np

DEBUG_TAPS = bool(os.environ.get("KTAPS"))

# ---------------- problem constants (hardcoded per the task spec) ----------
B = 32                  # graphs
LIG_PER, POC_PER = 64, 128
NDIMS, ATOM_NF, CTX_NF = 3, 16, 21
HIDDEN, EDGE_NF = 64, 8
NODE_NF = HIDDEN + 1
N_LAYERS, INV_SUB = 4, 2
NORM_FACTOR, NORM_CONST = 100.0, 1.0
COORDS_RANGE = 15.0 / N_LAYERS
N_CORES = 8
GPC = B // N_CORES      # graphs per core = 4
NLc = GPC * LIG_PER     # 256 lig nodes / core
NPc = GPC * POC_PER     # 512 pocket nodes / core
KSC = 100.0 / COORDS_RANGE  # sqrt-scale fold so recip gives CR/100/(sqrt+1)

_CACHE = {}


def _silu(x):
    return x / (1.0 + np.exp(-x))


# ======================================================================
# host-side parameter prep
# ======================================================================
def _prep_params(params):
    """Flatten + transform the nested param dict into per-core DRAM inputs."""
    p = {}

    def f32(x):
        return np.ascontiguousarray(np.asarray(x), dtype=np.float32)

    def bf16(x):
        import ml_dtypes
        return np.ascontiguousarray(np.asarray(x, dtype=np.float32)).astype(
            ml_dtypes.bfloat16
        )

    # encoders (feature-major pipelines)
    ae, ce, ad = params["atom_enc"], params["ctx_enc"], params["atom_dec"]
    p["ae_w1"] = f32(ae["w1"])            # [16, 32]
    p["ae_b1"] = f32(ae["b1"])[:, None]   # [32, 1]
    p["ae_w2"] = f32(ae["w2"])            # [32, 64]
    p["ae_b2"] = f32(ae["b2"])[:, None]   # [64, 1]
    p["ce_w1"] = f32(ce["w1"])            # [21, 42]
    p["ce_b1"] = f32(ce["b1"])[:, None]
    p["ce_w2"] = f32(ce["w2"])            # [42, 64]
    p["ce_b2"] = f32(ce["b2"])[:, None]
    p["ad_w1"] = f32(ad["w1"])            # [64, 32]
    p["ad_b1"] = f32(ad["b1"])[:, None]
    p["ad_w2"] = f32(ad["w2"])            # [32, 16]
    p["ad_b2"] = f32(ad["b2"])[:, None]

    edge_emb = np.asarray(params["edge_emb"], np.float32)  # [2, 8]

    for st, eattr_idx in (("ll", 1), ("lp", 0)):
        sp = params["egnn" if st == "ll" else "xatt"]
        ev = edge_emb[eattr_idx]  # [8]
        p[f"{st}_emb_w"] = f32(sp["emb"]["w"])              # [65, 64]
        p[f"{st}_emb_b"] = f32(sp["emb"]["b"])[:, None]     # [64, 1]
        if st == "lp":
            p["lp_kv_w"] = f32(sp["emb_kv"]["w"])           # [64, 64]
            p["lp_kv_b"] = f32(sp["emb_kv"]["b"])[:, None]
        p[f"{st}_out_w"] = f32(sp["out"]["w"])              # [64, 65]
        p[f"{st}_out_b"] = f32(sp["out"]["b"])[:, None]     # [65, 1]

        for bi, blk in enumerate(sp["blocks"]):
            units = [(f"g{gi}", g) for gi, g in enumerate(blk["gcls"])]
            units.append(("c", blk["coord"]))
            for uname, up in units:
                pre = f"{st}_b{bi}_{uname}"
                if uname == "c":
                    w1 = f32(up["w1"]); b1 = f32(up["b1"])
                    w2 = f32(up["w2"]); b2 = f32(up["b2"])
                else:
                    w1 = f32(up["e_w1"]); b1 = f32(up["e_b1"])
                    w2 = f32(up["e_w2"]); b2 = f32(up["e_b2"])
                Wa, Wb = w1[:64], w1[64:128]
                u, v = w1[128], w1[129]
                c0 = ev @ w1[130:138] + b1  # folded const [64]
                p[f"{pre}_wa"] = Wa                     # [64, 64] f32
                p[f"{pre}_wb"] = Wb
                for gp in range(2):
                    uvc = np.zeros((10, 128), np.float32)
                    uvc[2 * gp, :64] = u; uvc[2 * gp + 1, 64:] = u
                    uvc[4 + 2 * gp, :64] = v; uvc[5 + 2 * gp, 64:] = v
                    uvc[8, :64] = c0; uvc[8, 64:] = c0
                    p[f"{pre}_uvc{gp}"] = bf16(uvc)
                w2bd = np.zeros((128, 128), np.float32)
                w2bd[:64, :64] = w2; w2bd[64:, 64:] = w2
                p[f"{pre}_w2bd"] = bf16(w2bd)
                p[f"{pre}_b2bd"] = f32(np.concatenate([b2, b2]))[:, None]
                if uname == "c":
                    w3 = f32(up["w3"])  # [64, 1]
                    w3bd = np.zeros((128, 2), np.float32)
                    w3bd[:64, 0] = w3[:, 0]; w3bd[64:, 1] = w3[:, 0]
                    p[f"{pre}_w3bd"] = bf16(w3bd)
                else:
                    p[f"{pre}_c0bd"] = f32(np.concatenate([c0, c0]))[:, None]
                    nw1 = f32(up["n_w1"])   # [128, 64]
                    p[f"{pre}_nw1h"] = nw1[:64]
                    p[f"{pre}_nw1a"] = nw1[64:] / NORM_FACTOR
                    p[f"{pre}_nb1"] = f32(up["n_b1"])[:, None]
                    p[f"{pre}_nw2"] = f32(up["n_w2"])
                    p[f"{pre}_nb2"] = f32(up["n_b2"])[:, None]

    # rhs1 static images: [74, F] = indicator rows + zero rad/d0 rows + ones row
    def rhs1_img(nkv):
        img = np.zeros((74, 64 * nkv), np.float32)
        img[:64] = np.repeat(np.eye(64, dtype=np.float32), nkv, axis=1)
        img[72] = 1.0
        return bf16(img)
    p["ind_ll"] = rhs1_img(LIG_PER)
    p["ind_lp"] = rhs1_img(POC_PER)
    p["ones3"] = np.ones((3, 1), np.float32)
    p["ones_r"] = np.ones((1, 64), np.float32)
    return p


# ======================================================================
# bass builder
# ======================================================================
def _build(nc):
    import concourse.mybir as mybir
    import concourse.tile as tile
    from concourse.masks import make_identity

    F32 = mybir.dt.float32
    BF16 = mybir.dt.bfloat16
    AF = mybir.ActivationFunctionType
    OP = mybir.AluOpType
    AX = mybir.AxisListType

    D = {}  # dram tensors

    def din(name, shape, dt=F32):
        D[name] = nc.dram_tensor(name, list(shape), dt, kind="ExternalInput")
        return D[name]

    # ---- inputs
    din("hlT", [ATOM_NF, NLc]); din("hpT", [CTX_NF, NPc])
    din("tl", [1, NLc])
    din("xlT", [3, NLc]); din("xpT", [3, NPc])
    din("ind_ll", [74, 64 * LIG_PER], BF16)
    din("ind_lp", [74, 64 * POC_PER], BF16)
    din("ones3", [3, 1]); din("ones_r", [1, 64])
    for n, sh in [("ae_w1", [16, 32]), ("ae_b1", [32, 1]), ("ae_w2", [32, 64]),
                  ("ae_b2", [64, 1]), ("ce_w1", [21, 42]), ("ce_b1", [42, 1]),
                  ("ce_w2", [42, 64]), ("ce_b2", [64, 1]), ("ad_w1", [64, 32]),
                  ("ad_b1", [32, 1]), ("ad_w2", [32, 16]), ("ad_b2", [16, 1]),
                  ("lp_kv_w", [64, 64]), ("lp_kv_b", [64, 1])]:
        din(n, sh)
    for st in ("ll", "lp"):
        din(f"{st}_emb_w", [65, 64]); din(f"{st}_emb_b", [64, 1])
        din(f"{st}_out_w", [64, 65]); din(f"{st}_out_b", [65, 1])
        for bi in range(N_LAYERS):
            for uname in ("g0", "g1", "c"):
                pre = f"{st}_b{bi}_{uname}"
                din(f"{pre}_wa", [64, 64]); din(f"{pre}_wb", [64, 64])
                din(f"{pre}_uvc0", [10, 128], BF16); din(f"{pre}_uvc1", [10, 128], BF16)
                din(f"{pre}_w2bd", [128, 128], BF16)
                din(f"{pre}_b2bd", [128, 1])
                if uname == "c":
                    din(f"{pre}_w3bd", [128, 2], BF16)
                else:
                    din(f"{pre}_c0bd", [128, 1])
                    din(f"{pre}_nw1h", [64, 64]); din(f"{pre}_nw1a", [64, 64])
                    din(f"{pre}_nb1", [64, 1]); din(f"{pre}_nw2", [64, 64])
                    din(f"{pre}_nb2", [64, 1])
    out_d = nc.dram_tensor("out", [NLc, 3 + ATOM_NF], F32, kind="ExternalOutput")
    TAPS = {}
    if DEBUG_TAPS:
        for tn, tsh, tdt in ([("d_hlt", [65, NLc], F32), ("d_h0ll", [64, NLc], F32),
                             ("d_radll", [128, 64], F32), ("d_rhs1ll", [74, 512], BF16),
                             ("d_aggll", [128, 64], F32), ("d_zps", [128, 512], F32),
                             ("d_s1", [128, 512], BF16), ("d_hg1ll", [64, NLc], F32),
                             ("d_x1ll", [128, 3], F32), ("d_phill", [2, 4096], BF16),
                             ("d_d0ll", [74, 512], BF16),
                             ("d_phipm", [128, 64], BF16), ("d_th", [128, 64], F32),
                             ("d_rs", [128, 64], F32), ("d_sc", [128, 64], F32),
                             ("d_t1", [128, 3], F32), ("d_t2", [128, 3], F32),
                             ("d_stsb", [64, 128], F32),
                             ("d_hll_end", [64, NLc], F32), ("d_hlp_end", [64, NLc], F32),
                             ("d_xll_end", [128, 3], F32), ("d_xlp_end", [128, 3], F32),
                             ("d_xlp_b0", [128, 3], F32), ("d_agglp", [128, 64], F32)]
                            + [(f"d_hll_{bi}_{gi}", [64, NLc], F32)
                               for bi in range(4) for gi in range(2)]
                            + [(f"d_aggll_{bi}_{gi}", [128, 64], F32)
                               for bi in range(4) for gi in range(2)]
                            + [("d_zps01", [128, 512], F32), ("d_s101", [128, 512], BF16),
                               ("d_m01", [128, 512], F32), ("d_m00", [128, 512], F32),
                               ("d_at01", [64, 128], BF16), ("d_bt01", [64, 128], BF16)]):
            TAPS[tn] = nc.dram_tensor(tn, tsh, tdt, kind="ExternalOutput")

    with tile.TileContext(nc) as tc:
        with (
            tc.tile_pool(name="const", bufs=1) as cp,
            tc.tile_pool(name="work", bufs=3) as wp,
            tc.tile_pool(name="hpool", bufs=3) as hp,
            tc.tile_pool(name="encp", bufs=1) as ep,
            tc.tile_pool(name="phc", bufs=1) as phr,
            tc.tile_pool(name="xpool", bufs=3) as xp,
            tc.tile_pool(name="dram", bufs=2, space="DRAM") as dp,
            tc.tile_pool(name="pz", bufs=2, space="PSUM") as pz,
            tc.tile_pool(name="pm", bufs=2, space="PSUM") as pm,
            tc.tile_pool(name="ps", bufs=3, space="PSUM") as ps,
        ):
            S = {}  # persistent sbuf tiles

            def tap(name, ap, psum=False):
                if not DEBUG_TAPS or name not in TAPS:
                    return
                if psum:
                    shp = list(TAPS[name].shape)
                    tt = wp.tile(shp, TAPS[name].dtype, tag="tapp", name="tt")
                    nc.vector.tensor_copy(tt[:], ap)
                    ap = tt[:]
                nc.sync.dma_start(TAPS[name][:], ap)

            def ctile(name, shape, dt=F32):
                t = cp.tile(list(shape), dt, tag=name)
                S[name] = t
                return t

            def load(name, dt=None):
                d = D[name]
                t = ctile(name, d.shape, dt or d.dtype)
                nc.sync.dma_start(t[:], d[:])
                return t

            # ---- constants / weights into SBUF
            ident = ctile("ident", [128, 128])
            make_identity(nc, ident[:])
            cb_ln = ctile("cb_ln", [1, 1])
            nc.vector.memset(cb_ln[:], 1e-5)
            cb_rad = ctile("cb_rad", [128, 1])
            nc.vector.memset(cb_rad[:], 1e-8 * KSC * KSC)
            identb = ctile("identb", [128, 128], BF16)
            nc.vector.tensor_copy(identb[:], ident[:])

            for n in D:
                if n not in ("ind_ll", "ind_lp"):
                    load(n)

            # ================= encoders (feature-major) =================
            def enc_mlp(w1, b1, w2, b2, src, n_nodes, f_in, f_mid):
                e1p = ps.tile([f_mid, n_nodes], F32, tag="psmall")
                nc.tensor.matmul(e1p[:], S[w1][:], src[:], start=True, stop=True)
                s1 = ep.tile([f_mid, n_nodes], F32, tag="enc_s1")
                nc.scalar.activation(s1[:], e1p[:], AF.Silu, bias=S[b1][:])
                e2p = ps.tile([64, n_nodes], F32, tag="psmall")
                nc.tensor.matmul(e2p[:], S[w2][:], s1[:], start=True, stop=True)
                # clip +-50
                emb = ep.tile([64, n_nodes], F32, tag="enc_emb")
                nc.scalar.activation(emb[:], e2p[:], AF.Identity, bias=S[b2][:])
                nc.vector.tensor_scalar(emb[:], emb[:], 50.0, -50.0, OP.min, OP.max)
                # layernorm over the 64 features (partition dim) via matmul
                ones_col = cp.tile([64, 2], F32, tag="ln_ones")
                nc.vector.memset(ones_col[:, 0:1], 1.0 / 64)
                nc.vector.memset(ones_col[:, 1:2], 0.0)
                sq = ep.tile([64, n_nodes], F32, tag="enc_sq")
                nc.vector.tensor_tensor(sq[:], emb[:], emb[:], OP.mult)
                mvp0 = ps.tile([1, n_nodes], F32, tag="psmall")
                nc.tensor.matmul(mvp0[:], ones_col[:, 0:1], emb[:], start=True, stop=True)
                mvp1 = ps.tile([1, n_nodes], F32, tag="psmall")
                nc.tensor.matmul(mvp1[:], ones_col[:, 0:1], sq[:], start=True, stop=True)
                mva = ep.tile([1, n_nodes], F32, tag="enc_mva")
                nc.vector.tensor_copy(mva[:], mvp0[:])
                mvb = ep.tile([1, n_nodes], F32, tag="enc_mvb")
                nc.vector.tensor_copy(mvb[:], mvp1[:])
                var = ep.tile([1, n_nodes], F32, tag="enc_var")
                nc.vector.tensor_tensor(var[:], mva[:], mva[:], OP.mult)
                nc.vector.tensor_tensor(var[:], mvb[:], var[:], OP.subtract)
                rstd = ep.tile([1, n_nodes], F32, tag="enc_rstd")
                nc.scalar.activation(rstd[:], var[:], AF.Sqrt, bias=S["cb_ln"][:])
                nc.vector.reciprocal(rstd[:], rstd[:])
                # broadcast mean/rstd across 64 partitions (gpsimd)
                mb = ep.tile([64, n_nodes], F32, tag="enc_mb")
                rb = ep.tile([64, n_nodes], F32, tag="enc_rb")
                nc.gpsimd.partition_broadcast(mb[:], mva[:])
                nc.gpsimd.partition_broadcast(rb[:], rstd[:])
                nc.vector.tensor_tensor(emb[:], emb[:], mb[:], OP.subtract)
                nc.vector.tensor_tensor(emb[:], emb[:], rb[:], OP.mult)
                return emb

            hl_emb = enc_mlp("ae_w1", "ae_b1", "ae_w2", "ae_b2", S["hlT"], NLc, 16, 32)
            hp_emb = enc_mlp("ce_w1", "ce_b1", "ce_w2", "ce_b2", S["hpT"], NPc, 21, 42)

            # h_l_t = [hl_emb ; t]  -> [65, 256]
            hlt = ctile("hlt", [65, NLc])
            nc.vector.tensor_copy(hlt[0:64, :], hl_emb[:])
            nc.vector.tensor_copy(hlt[64:65, :], S["tl"][:])
            tap("d_hlt", hlt[:])

            # ================= geometry state =================
            # x_pm per graph-pair [128, 3]; x_fm [6, n-per-graph]
            def transpose_to_pm(src_a, src_b, gp):
                """two [3, 64] fm tiles -> packed pair-major [128, 3]"""
                t = xp.tile([128, 3], F32, tag=f"xpm_t_{gp}")
                pp = ps.tile([64, 3], F32, tag="psmall")
                nc.tensor.transpose(pp[:], src_a[:], ident[:3, :3])
                nc.vector.tensor_copy(t[0:64, :], pp[:])
                pp2 = ps.tile([64, 3], F32, tag="psmall")
                nc.tensor.transpose(pp2[:], src_b[:], ident[:3, :3])
                nc.vector.tensor_copy(t[64:128, :], pp2[:])
                return t

            # lig x_fm per gp: [6, 64]
            x0l_fm, x0l_pm = [], []
            for gp in range(2):
                xfa = ctile(f"x0l_fma_{gp}", [3, 64])
                xfb = ctile(f"x0l_fmb_{gp}", [3, 64])
                nc.vector.tensor_copy(xfa[:], S["xlT"][:, (2 * gp) * 64:(2 * gp + 1) * 64])
                nc.vector.tensor_copy(xfb[:], S["xlT"][:, (2 * gp + 1) * 64:(2 * gp + 2) * 64])
                x0l_fm.append((xfa, xfb))
                x0l_pm.append(transpose_to_pm(xfa, xfb, f"x0_{gp}"))
            # pocket x (static): fm [6, 128] per gp; pm per graph [128, 3] -> per gp two [128,3]? need xk_pm per graph [nkv,3]
            xp_fm, xp_pm = [], []
            for gp in range(2):
                xfa = ctile(f"xp_fma_{gp}", [3, POC_PER])
                xfb = ctile(f"xp_fmb_{gp}", [3, POC_PER])
                nc.vector.tensor_copy(xfa[:], S["xpT"][:, (2 * gp) * 128:(2 * gp + 1) * 128])
                nc.vector.tensor_copy(xfb[:], S["xpT"][:, (2 * gp + 1) * 128:(2 * gp + 2) * 128])
                xp_fm.append((xfa, xfb))
                pms = []
                for g, xf in enumerate((xfa, xfb)):
                    t = ctile(f"xp_pm_{gp}_{g}", [128, 3])
                    pp = ps.tile([128, 3], F32, tag="psmall")
                    nc.tensor.transpose(pp[:], xf[:], ident[:3, :3])
                    nc.vector.tensor_copy(t[:], pp[:])
                    pms.append(t)
                xp_pm.append(pms)

            # ================= message-passing stacks =================
            results = {}

            def run_stack(st):
                cross = st == "lp"
                NKV = POC_PER if cross else LIG_PER
                Fg = 64 * NKV                   # pairs per graph
                ICH = 512 // NKV                # i's per 512-chunk
                NCH = Fg // 512                 # chunks per graph-pair
                ind_name = "ind_lp" if cross else "ind_ll"

                # h0 = h_l_t @ emb_w + emb_b   [64, 256]
                h0p = ps.tile([64, NLc], F32, tag="psmall")
                nc.tensor.matmul(h0p[:], S[f"{st}_emb_w"][:], hlt[:], start=True, stop=True)
                h = hp.tile([64, NLc], F32, tag=f"h_{st}")
                nc.scalar.activation(h[:], h0p[:], AF.Identity, bias=S[f"{st}_emb_b"][:])
                if st == "ll":
                    tap("d_h0ll", h[:])

                if cross:
                    hkp = ps.tile([64, NPc], F32, tag="psmall")
                    nc.tensor.matmul(hkp[:], S["lp_kv_w"][:], hp_emb[:], start=True, stop=True)
                    hk = ctile("hk_lp", [64, NPc])
                    nc.scalar.activation(hk[:], hkp[:], AF.Identity, bias=S["lp_kv_b"][:])

                # rhs1 (shared): [74, Fg] bf16; rows: 0-63 indicator,
                # 64-67 rad gp0/gp1, 68-71 d0 gp0/gp1, 72 ones, 73 zero
                rhs1 = ctile(f"rhs1_{st}", [74, Fg], BF16)
                nc.sync.dma_start(rhs1[:], D[ind_name][:])

                # gram machinery per gp (persistent tiles)


                def gram_rad(gp, xq_fm, xq_pm, xk_fm, rows_lo, f32_out):
                    """rad for graph-pair gp -> (rad_pm f32 [128, NKV]).
                    Also writes bf16 flatten into rhs1 rows [rows_lo, rows_lo+2)."""
                    xqa, xqb = xq_fm
                    xka, xkb = xk_fm
                    gpp = ps.tile([128, NKV], F32, tag="psmall")
                    for half, (xq_h, xk_h) in enumerate(((xqa, xka), (xqb, xkb))):
                        glh = wp.tile([3, 64], F32, tag="glh")
                        nc.vector.tensor_scalar_mul(glh[:], xq_h[:], -2.0)
                        sqk = wp.tile([3, NKV], F32, tag=f"sqk_{st}")
                        nc.vector.tensor_tensor(sqk[:], xk_h[:], xk_h[:], OP.mult)
                        q2p = ps.tile([1, NKV], F32, tag="psmall")
                        nc.tensor.matmul(q2p[:], S["ones3"][:], sqk[:], start=True, stop=True)
                        q2 = wp.tile([1, NKV], F32, tag="q2")
                        nc.vector.tensor_copy(q2[:], q2p[:])
                        lo_p = 64 * half
                        nc.tensor.matmul(gpp[lo_p:lo_p + 64, :], glh[:], xk_h[:],
                                         start=True, stop=False)
                        nc.tensor.matmul(gpp[lo_p:lo_p + 64, :], S["ones_r"][:], q2[:],
                                         start=False, stop=True)
                    # |x_i|^2 per-partition bias
                    xsq = wp.tile([128, 3], F32, tag="xsq")
                    nc.vector.tensor_tensor(xsq[:], xq_pm[:], xq_pm[:], OP.mult)
                    xq2 = wp.tile([128, 1], F32, tag="xq2")
                    nc.vector.tensor_reduce(xq2[:], xsq[:], AX.X, OP.add)
                    radf = None
                    if f32_out:
                        radf = wp.tile([128, NKV], F32, tag=f"radf_{st}", name="radf")
                    radb = wp.tile([128, NKV], BF16, tag=f"radb_{st}")
                    nc.scalar.activation(radb[:], gpp[:], AF.Identity, bias=xq2[:])
                    if f32_out:
                        nc.scalar.activation(radf[:], gpp[:], AF.Identity, bias=xq2[:])
                        nc.vector.tensor_scalar_max(radf[:], radf[:], 0.0)
                    lo = rows_lo + 2 * gp
                    db = dp.tile([128, NKV], BF16, tag=f"radd_{st}")
                    nc.sync.dma_start(db[:], radb[:])
                    nc.sync.dma_start(
                        rhs1[lo:lo + 2, :],
                        db[:].rearrange("(two i) j -> two (i j)", two=2))
                    return radf

                def make_AT(hsrc, w_name, tagpre, gp, n_m):
                    """A_T [n_m, 128] psum from feature-major h [64, ...]."""
                    atp = ps.tile([n_m, 128], F32, tag="psmall")
                    c0 = (2 * gp) * n_m
                    nc.tensor.matmul(atp[:, 0:64], hsrc[:, c0:c0 + n_m],
                                     S[w_name][:], start=True, stop=True)
                    nc.tensor.matmul(atp[:, 64:128], hsrc[:, c0 + n_m:c0 + 2 * n_m],
                                     S[w_name][:], start=True, stop=True)
                    return atp

                # initial geometry per stack
                xq_fm = [x0l_fm[0], x0l_fm[1]]
                xq_pm = [x0l_pm[0], x0l_pm[1]]
                if cross:
                    xk_fm = xp_fm
                else:
                    xk_fm = xq_fm

                # d0 rows (once per stack)
                for gp in range(2):
                    gram_rad(gp, xq_fm[gp], xq_pm[gp], xk_fm[gp], 68, False)
                if st == "ll":
                    tap("d_d0ll", rhs1[:, 0:512])

                bt_cache = {}

                for bi in range(N_LAYERS):
                    # ---- block prologue: rad + coord scalars per gp
                    rs_all = []
                    for gp in range(2):
                        radf = gram_rad(gp, xq_fm[gp], xq_pm[gp], xk_fm[gp], 64, True)
                        if st == "ll" and bi == 0 and gp == 0:
                            tap("d_radll", radf[:])
                            tap("d_rhs1ll", rhs1[:, 0:512])
                        den = wp.tile([128, NKV], F32, tag=f"den_{st}")
                        nc.scalar.activation(den[:], radf[:], AF.Sqrt,
                                             bias=S["cb_rad"][:], scale=KSC * KSC)
                        nc.vector.tensor_scalar_add(den[:], den[:], KSC)
                        nc.vector.reciprocal(den[:], den[:])
                        rs_all.append(den)

                    def pair_mlp(pre, gp, at_sb, bt_sb, want_m):
                        """Run chunked pair MLP. Returns agg [128, 64] (if want_m)
                        else phi_row [2, Fg]."""
                        # lhs1 = [A_T ; uvc] bf16
                        lhs1 = wp.tile([74, 128], BF16, tag="lhs1")
                        nc.vector.tensor_copy(lhs1[0:64, :], at_sb[:])
                        nc.vector.tensor_copy(lhs1[64:74, :], S[f"{pre}_uvc{gp}"][:])
                        agg = None
                        phi_row = None
                        if want_m:
                            agg = wp.tile([128, 64], F32, tag="agg", name="agg")
                        else:
                            phi_row = phr.tile([2, Fg], BF16, tag=f"phi_row_{st}",
                                              name="phi_row")
                        for ci in range(NCH):
                            zps = pz.tile([128, 512], F32, tag="zchunk")
                            nc.tensor.matmul(zps[:], lhs1[:],
                                             rhs1[:, ci * 512:(ci + 1) * 512],
                                             start=True, stop=False)
                            rhsb = identb[:NKV, None, :NKV].to_broadcast(
                                (NKV, ICH, NKV))
                            nc.tensor.matmul(zps[:].rearrange("p (i j) -> p i j", i=ICH),
                                             bt_sb[:], rhsb, start=False, stop=True)
                            if (DEBUG_TAPS and st == "ll" and ci == 0 and gp == 0
                                    and pre.endswith("b0_g0")):
                                tap("d_zps", zps[:], psum=True)
                            s1 = wp.tile([128, 512], BF16, tag="s1")
                            nc.scalar.activation(s1[:], zps[:], AF.Silu)
                            if (DEBUG_TAPS and st == "ll" and ci == 0 and gp == 0
                                    and pre.endswith("b0_g0")):
                                tap("d_s1", s1[:])
                            if (DEBUG_TAPS and st == "ll" and ci == 0 and gp == 0
                                    and pre.endswith("b0_g1")):
                                tap("d_s101", s1[:])
                            mps = pm.tile([128, 512], F32, tag="mchunk")
                            nc.tensor.matmul(mps[:], S[f"{pre}_w2bd"][:], s1[:],
                                             start=True, stop=True)
                            if want_m:
                                mt = wp.tile([128, 512], F32, tag="mt")
                                nc.scalar.activation(mt[:], mps[:], AF.Silu,
                                                     bias=S[f"{pre}_b2bd"][:])
                                if (DEBUG_TAPS and st == "ll" and ci == 0 and gp == 0
                                        and bi == 0):
                                    tap(f"d_m0{gi}", mt[:])
                                nc.vector.tensor_reduce(
                                    agg[:, ci * ICH:(ci + 1) * ICH],
                                    mt[:].rearrange("p (i j) -> p i j", i=ICH),
                                    AX.X, OP.add)
                            else:
                                s2 = wp.tile([128, 512], BF16, tag="s2")
                                nc.scalar.activation(s2[:], mps[:], AF.Silu,
                                                     bias=S[f"{pre}_b2bd"][:])
                                php = ps.tile([2, 512], F32, tag="psmall")
                                nc.tensor.matmul(php[:], S[f"{pre}_w3bd"][:], s2[:],
                                                 start=True, stop=True)
                                nc.vector.tensor_copy(
                                    phi_row[:, ci * 512:(ci + 1) * 512], php[:])
                        return agg if want_m else phi_row

                    # ---- gcl units
                    for gi in range(INV_SUB):
                        pre = f"{st}_b{bi}_g{gi}"
                        aggs = []
                        for gp in range(2):
                            atp = make_AT(h, f"{pre}_wa", "at", gp, 64)
                            at_sb = wp.tile([64, 128], BF16, tag="at_sb")
                            nc.vector.tensor_copy(at_sb[:], atp[:])
                            if st == "ll" and bi == 0 and gi == 1 and gp == 0:
                                tap("d_at01", at_sb[:])
                            if cross:
                                key = (bi, gi, gp)
                                if key not in bt_cache:
                                    btp = make_AT(hk, f"{pre}_wb", "bt", gp, POC_PER)
                                    bt_sb = ctile(f"btc_{st}_{bi}_{gi}_{gp}",
                                                  [POC_PER, 128], BF16)
                                    nc.vector.tensor_copy(bt_sb[:], btp[:])
                                    bt_cache[key] = bt_sb
                                bt_sb = bt_cache[key]
                            else:
                                btp = make_AT(h, f"{pre}_wb", "bt", gp, 64)
                                bt_sb = wp.tile([64, 128], BF16, tag="bt_sb")
                                nc.vector.tensor_copy(bt_sb[:], btp[:])
                                if st == "ll" and bi == 0 and gi == 1 and gp == 0:
                                    tap("d_bt01", bt_sb[:])
                            agg = pair_mlp(pre, gp, at_sb, bt_sb, True)
                            if not cross:
                                # diagonal correction: subtract m_ii
                                zd = wp.tile([64, 128], BF16, tag="zd")
                                nc.vector.tensor_tensor(zd[:], at_sb[:], bt_sb[:], OP.add)
                                zdp = ps.tile([128, 64], BF16, tag="psmall")
                                nc.tensor.transpose(zdp[:], zd[:], identb[:64, :64])
                                s1d = wp.tile([128, 64], BF16, tag="s1d")
                                nc.scalar.activation(s1d[:], zdp[:], AF.Silu,
                                                     bias=S[f"{pre}_c0bd"][:])
                                mdp = ps.tile([128, 64], F32, tag="psmall")
                                nc.tensor.matmul(mdp[:], S[f"{pre}_w2bd"][:], s1d[:],
                                                 start=True, stop=True)
                                md = wp.tile([128, 64], F32, tag="md")
                                nc.scalar.activation(md[:], mdp[:], AF.Silu,
                                                     bias=S[f"{pre}_b2bd"][:])
                                nc.vector.tensor_tensor(agg[:], agg[:], md[:], OP.subtract)
                            if st == "ll" and bi == 0 and gi == 0 and gp == 0:
                                tap("d_aggll", agg[:])
                            if st == "ll" and gp == 0:
                                tap(f"d_aggll_{bi}_{gi}", agg[:])
                            if st == "lp" and bi == 0 and gi == 0 and gp == 0:
                                tap("d_agglp", agg[:])
                            aggs.append(agg)
                        # node update (all 4 graphs)
                        agg_all = wp.tile([64, NLc], F32, tag="agg_all")
                        for gp in range(2):
                            nc.vector.tensor_copy(
                                agg_all[:, (2 * gp) * 64:(2 * gp + 1) * 64],
                                aggs[gp][0:64, :])
                            nc.vector.tensor_copy(
                                agg_all[:, (2 * gp + 1) * 64:(2 * gp + 2) * 64],
                                aggs[gp][64:128, :])
                        np1 = ps.tile([64, NLc], F32, tag="psmall")
                        nc.tensor.matmul(np1[:], S[f"{pre}_nw1h"][:], h[:],
                                         start=True, stop=False)
                        nc.tensor.matmul(np1[:], S[f"{pre}_nw1a"][:], agg_all[:],
                                         start=False, stop=True)
                        sn = wp.tile([64, NLc], F32, tag="sn")
                        nc.scalar.activation(sn[:], np1[:], AF.Silu, bias=S[f"{pre}_nb1"][:])
                        np2 = ps.tile([64, NLc], F32, tag="psmall")
                        nc.tensor.matmul(np2[:], S[f"{pre}_nw2"][:], sn[:],
                                         start=True, stop=True)
                        h_new = hp.tile([64, NLc], F32, tag=f"h_{st}")
                        nc.vector.scalar_tensor_tensor(h_new[:], np2[:], S[f"{pre}_nb2"][:],
                                                       h[:], OP.add, OP.add)
                        h = h_new
                        if st == "ll" and bi == 0 and gi == 0:
                            tap("d_hg1ll", h[:])
                        if st == "ll":
                            tap(f"d_hll_{bi}_{gi}", h[:])

                    # ---- coord unit
                    pre = f"{st}_b{bi}_c"
                    new_xq_fm, new_xq_pm = [], []
                    for gp in range(2):
                        atp = make_AT(h, f"{pre}_wa", "at", gp, 64)
                        at_sb = wp.tile([64, 128], BF16, tag="at_sb")
                        nc.vector.tensor_copy(at_sb[:], atp[:])
                        if cross:
                            key = ("c", bi, gp)
                            if key not in bt_cache:
                                btp = make_AT(hk, f"{pre}_wb", "bt", gp, POC_PER)
                                bt_sb = ctile(f"btc_{st}_c{bi}_{gp}", [POC_PER, 128], BF16)
                                nc.vector.tensor_copy(bt_sb[:], btp[:])
                                bt_cache[key] = bt_sb
                            bt_sb = bt_cache[key]
                        else:
                            btp = make_AT(h, f"{pre}_wb", "bt", gp, 64)
                            bt_sb = wp.tile([64, 128], BF16, tag="bt_sb")
                            nc.vector.tensor_copy(bt_sb[:], btp[:])
                        phi_row = pair_mlp(pre, gp, at_sb, bt_sb, False)
                        # phi -> pair-major
                        phi_pm = wp.tile([128, NKV], BF16, tag=f"phi_pm_{st}")
                        db2 = dp.tile([2, Fg], BF16, tag=f"phid_{st}")
                        nc.sync.dma_start(db2[:], phi_row[:])
                        nc.sync.dma_start(
                            phi_pm[:],
                            db2[:].rearrange("two (i j) -> (two i) j", i=64))
                        if st == "ll" and bi == 0 and gp == 0:
                            tap("d_phill", phi_row[:])
                        th = wp.tile([128, NKV], F32, tag=f"th_{st}")
                        nc.scalar.activation(th[:], phi_pm[:], AF.Tanh)
                        sc = wp.tile([128, NKV], F32, tag=f"sc_{st}")
                        nc.vector.tensor_tensor(sc[:], th[:], rs_all[gp][:], OP.mult)
                        if st == "ll" and bi == 0 and gp == 0:
                            tap("d_phipm", phi_pm[:])
                            tap("d_th", th[:])
                            tap("d_rs", rs_all[gp][:])
                            tap("d_sc", sc[:])
                        # term1 = x_pm * rowsum(sc)
                        srow = wp.tile([128, 1], F32, tag="srow")
                        nc.vector.tensor_reduce(srow[:], sc[:], AX.X, OP.add)
                        t1 = wp.tile([128, 3], F32, tag="t1")
                        nc.vector.tensor_tensor(
                            t1[:], xq_pm[gp][:],
                            srow[:, 0:1].to_broadcast((128, 3)), OP.mult)
                        # term2 = sum_j sc[i,j] * xk_pm[j,:]
                        stp = ps.tile([NKV, 128], F32, tag="psmall")
                        nc.tensor.transpose(stp[:], sc[:], ident[:])
                        st_sb = wp.tile([NKV, 128], F32, tag=f"st_sb_{st}")
                        nc.vector.tensor_copy(st_sb[:], stp[:])
                        t2p = ps.tile([128, 3], F32, tag="psmall")
                        if cross:
                            xk_pm_A, xk_pm_B = xp_pm[gp][0], xp_pm[gp][1]
                        else:
                            xk_pm_A, xk_pm_B = None, None  # self: from xq_pm halves
                        if cross:
                            nc.tensor.matmul(t2p[0:64, :], st_sb[:, 0:64], xk_pm_A[:],
                                             start=True, stop=True)
                            nc.tensor.matmul(t2p[64:128, :], st_sb[:, 64:128], xk_pm_B[:],
                                             start=True, stop=True,
                                             tile_position=(0, 64))
                        else:
                            xkA = wp.tile([64, 3], F32, tag="xkA")
                            nc.vector.tensor_copy(xkA[:], xq_pm[gp][0:64, :])
                            xkB = wp.tile([64, 3], F32, tag="xkB")
                            nc.vector.tensor_copy(xkB[:], xq_pm[gp][64:128, :])
                            nc.tensor.matmul(t2p[0:64, :], st_sb[:, 0:64], xkA[:],
                                             start=True, stop=True)
                            nc.tensor.matmul(t2p[64:128, :], st_sb[:, 64:128], xkB[:],
                                             start=True, stop=True,
                                             tile_position=(0, 64))
                        if st == "ll" and bi == 0 and gp == 0:
                            tap("d_t1", t1[:])
                            tap("d_t2", t2p[:], psum=True)
                            tap("d_stsb", st_sb[:])
                        x_new = xp.tile([128, 3], F32, tag=f"xpm_{st}_{gp}")
                        nc.vector.tensor_tensor(x_new[:], xq_pm[gp][:], t1[:], OP.add)
                        nc.vector.tensor_tensor(x_new[:], x_new[:], t2p[:], OP.subtract)
                        new_xq_pm.append(x_new)
                        if st == "ll" and bi == 0 and gp == 0:
                            tap("d_x1ll", x_new[:])
                        if st == "lp" and bi == 0 and gp == 0:
                            tap("d_xlp_b0", x_new[:])
                        # back to feature-major (two [3, 64] tiles)
                        xfa = xp.tile([3, 64], F32, tag=f"xfma_{st}_{gp}")
                        fp1 = ps.tile([3, 64], F32, tag="psmall")
                        nc.tensor.transpose(fp1[:], x_new[0:64, :], ident[:64, :64])
                        nc.vector.tensor_copy(xfa[:], fp1[:])
                        xfb = xp.tile([3, 64], F32, tag=f"xfmb_{st}_{gp}")
                        fp2 = ps.tile([3, 64], F32, tag="psmall")
                        nc.tensor.transpose(fp2[:], x_new[64:128, :], ident[64:128, 64:128])
                        nc.vector.tensor_copy(xfb[:], fp2[:])
                        new_xq_fm.append((xfa, xfb))
                    xq_fm, xq_pm = new_xq_fm, new_xq_pm
                    if not cross:
                        xk_fm = xq_fm

                tap(f"d_h{st}_end", h[:])
                tap(f"d_x{st}_end", xq_pm[0][:])
                # stack epilogue: h65 = h @ out_w + out_b
                h65p = ps.tile([65, NLc], F32, tag="psmall")
                nc.tensor.matmul(h65p[:], S[f"{st}_out_w"][:], h[:], start=True, stop=True)
                h65 = ctile(f"h65_{st}", [65, NLc])
                nc.scalar.activation(h65[:], h65p[:], AF.Identity, bias=S[f"{st}_out_b"][:])
                results[st] = (h65, xq_pm)

            run_stack("ll")
            run_stack("lp")

            # ================= final combine =================
            h65_ll, x_ll_pm = results["ll"]
            h65_lp, x_lp_pm = results["lp"]
            hfin = cp.tile([64, NLc], F32, tag="hfin")
            tmp = wp.tile([64, NLc], F32, tag="cmb_tmp")
            nc.vector.tensor_scalar_mul(tmp[:], h65_lp[0:64, :], 0.4)
            nc.vector.scalar_tensor_tensor(hfin[:], h65_ll[0:64, :], 0.6, tmp[:],
                                           OP.mult, OP.add)
            # feats = atom_dec MLP
            f1p = ps.tile([32, NLc], F32, tag="psmall")
            nc.tensor.matmul(f1p[:], S["ad_w1"][:], hfin[:], start=True, stop=True)
            f1 = wp.tile([32, NLc], F32, tag="f1")
            nc.scalar.activation(f1[:], f1p[:], AF.Silu, bias=S["ad_b1"][:])
            f2p = ps.tile([16, NLc], F32, tag="psmall")
            nc.tensor.matmul(f2p[:], S["ad_w2"][:], f1[:], start=True, stop=True)
            feats = cp.tile([16, NLc], F32, tag="feats")
            nc.scalar.activation(feats[:], f2p[:], AF.Identity, bias=S["ad_b2"][:])
            velf = cp.tile([3, NLc], F32, tag="velf")
            # vel rows: per gp compute vel_pm then transpose into outf rows 0-2
            for gp in range(2):
                d1 = wp.tile([128, 3], F32, tag="veld1")
                nc.vector.tensor_tensor(d1[:], x_ll_pm[gp][:], x0l_pm[gp][:], OP.subtract)
                d2 = wp.tile([128, 3], F32, tag="veld2")
                nc.vector.tensor_tensor(d2[:], x_lp_pm[gp][:], x0l_pm[gp][:], OP.subtract)
                nc.vector.tensor_scalar_mul(d2[:], d2[:], 0.4)
                vel = wp.tile([128, 3], F32, tag="vel")
                nc.vector.scalar_tensor_tensor(vel[:], d1[:], 0.6, d2[:], OP.mult, OP.add)
                vp1 = ps.tile([3, 64], F32, tag="psmall")
                nc.tensor.transpose(vp1[:], vel[0:64, :], ident[:64, :64])
                nc.vector.tensor_copy(velf[:, (2 * gp) * 64:(2 * gp + 1) * 64], vp1[:])
                vp2 = ps.tile([3, 64], F32, tag="psmall")
                nc.tensor.transpose(vp2[:], vel[64:128, :], ident[64:128, 64:128])
                nc.vector.tensor_copy(velf[:, (2 * gp + 1) * 64:(2 * gp + 2) * 64], vp2[:])
            # transpose [vel | feats] -> out [256, 19] (free-dim concat in ot)
            for half in range(2):
                ot = wp.tile([128, 19], F32, tag="ot")
                opv = ps.tile([128, 3], F32, tag="psmall")
                nc.tensor.transpose(opv[:], velf[:, half * 128:(half + 1) * 128],
                                    ident[:3, :3])
                nc.vector.tensor_copy(ot[:, 0:3], opv[:])
                opf = ps.tile([128, 16], F32, tag="psmall")
                nc.tensor.transpose(opf[:], feats[:, half * 128:(half + 1) * 128],
                                    ident[:16, :16])
                nc.vector.tensor_copy(ot[:, 3:19], opf[:])
                nc.sync.dma_start(out_d[half * 128:(half + 1) * 128, :], ot[:])

    nc.compile()
    return nc


# ======================================================================
# public entry
# ======================================================================
def kernel(xh_lig, xh_context, t, mask_lig, mask_context, edges_ll, edges_lp,
           params):
    import jax

    xh_lig = np.asarray(xh_lig, np.float32)
    xh_context = np.asarray(xh_context, np.float32)
    t_np = np.asarray(t, np.float32)
    ml = np.asarray(mask_lig)

    # deterministic reference noise (key 42), computed on CPU
    cpu = jax.local_devices(backend="cpu")[0]
    with jax.default_device(cpu):
        k1, k2 = jax.random.split(jax.random.key(42))
        nl = 1e-3 * np.asarray(jax.random.normal(k1, (xh_lig.shape[0], NDIMS),
                                                 jax.numpy.float32))
        npk = 1e-3 * np.asarray(jax.random.normal(k2, (xh_context.shape[0], NDIMS),
                                                  jax.numpy.float32))
    x_l = xh_lig[:, :NDIMS] + nl
    x_p = xh_context[:, :NDIMS] + npk
    h_l = xh_lig[:, NDIMS:]
    h_p = xh_context[:, NDIMS:]
    t_lig = t_np[ml, 0]  # [2048]

    if "nc" not in _CACHE:
        import concourse.bacc as bacc
        nc = bacc.Bacc(None, target_bir_lowering=False)
        _CACHE["nc"] = _build(nc)
    nc = _CACHE["nc"]

    if "pp" not in _CACHE:
        _CACHE["pp"] = _prep_params(params)
    pp = _CACHE["pp"]

    in_maps = []
    for c in range(N_CORES):
        ls, le = c * NLc, (c + 1) * NLc
        ps_, pe = c * NPc, (c + 1) * NPc
        m = dict(pp)
        m["hlT"] = np.ascontiguousarray(h_l[ls:le].T)
        m["hpT"] = np.ascontiguousarray(h_p[ps_:pe].T)
        m["tl"] = np.ascontiguousarray(t_lig[ls:le])[None, :]
        m["xlT"] = np.ascontiguousarray(x_l[ls:le].T)
        m["xpT"] = np.ascontiguousarray(x_p[ps_:pe].T)
        in_maps.append(m)

    from concourse.bass_utils import run_bass_kernel_spmd
    res = run_bass_kernel_spmd(nc, in_maps, core_ids=list(range(N_CORES)),
                               trace=bool(os.environ.get("KTRACE")))
    _CACHE["last_results"] = res

    lig_update = np.concatenate([res.results[c]["out"] for c in range(N_CORES)],
                                axis=0).astype(np.float32)
    pocket_update = np.zeros_like(xh_context)
    return lig_update, pocket_update


# revision 26
# speedup vs baseline: 1.1628x; 1.1628x over previous
"""AtomicaDynamics EGNN/cross-attention message passing on 8 TRN2 NeuronCores.

Data-parallel over graphs (4 graphs/core, edges never cross graphs).
Dense all-pairs formulation: per graph, ll messages over the complete 64x64
grid (diagonal corrected), lp messages over the complete 64x128 bipartite
grid. Two graphs are packed onto the 128 SBUF partitions (feature-major pair
tensors), with block-diagonal weights.
"""
import sys

sys.path.insert(0, "/opt/trn_rl_repo")

import os
import numpy as np

DEBUG_TAPS = bool(os.environ.get("KTAPS"))

# ---------------- problem constants (hardcoded per the task spec) ----------
B = 32                  # graphs
LIG_PER, POC_PER = 64, 128
NDIMS, ATOM_NF, CTX_NF = 3, 16, 21
HIDDEN, EDGE_NF = 64, 8
NODE_NF = HIDDEN + 1
N_LAYERS, INV_SUB = 4, 2
NORM_FACTOR, NORM_CONST = 100.0, 1.0
COORDS_RANGE = 15.0 / N_LAYERS
N_CORES = 8
GPC = B // N_CORES      # graphs per core = 4
NLc = GPC * LIG_PER     # 256 lig nodes / core
NPc = GPC * POC_PER     # 512 pocket nodes / core
KSC = 100.0 / COORDS_RANGE  # sqrt-scale fold so recip gives CR/100/(sqrt+1)

_CACHE = {}


def _silu(x):
    return x / (1.0 + np.exp(-x))


# ======================================================================
# host-side parameter prep
# ======================================================================
def _prep_params(params):
    """Flatten + transform the nested param dict into per-core DRAM inputs."""
    p = {}

    def f32(x):
        return np.ascontiguousarray(np.asarray(x), dtype=np.float32)

    def bf16(x):
        import ml_dtypes
        return np.ascontiguousarray(np.asarray(x, dtype=np.float32)).astype(
            ml_dtypes.bfloat16
        )

    # encoders (feature-major pipelines)
    ae, ce, ad = params["atom_enc"], params["ctx_enc"], params["atom_dec"]
    p["ae_w1"] = f32(ae["w1"])            # [16, 32]
    p["ae_b1"] = f32(ae["b1"])[:, None]   # [32, 1]
    p["ae_w2"] = f32(ae["w2"])            # [32, 64]
    p["ae_b2"] = f32(ae["b2"])[:, None]   # [64, 1]
    p["ce_w1"] = f32(ce["w1"])            # [21, 42]
    p["ce_b1"] = f32(ce["b1"])[:, None]
    p["ce_w2"] = f32(ce["w2"])            # [42, 64]
    p["ce_b2"] = f32(ce["b2"])[:, None]
    p["ad_w1"] = f32(ad["w1"])            # [64, 32]
    p["ad_b1"] = f32(ad["b1"])[:, None]
    p["ad_w2"] = f32(ad["w2"])            # [32, 16]
    p["ad_b2"] = f32(ad["b2"])[:, None]

    edge_emb = np.asarray(params["edge_emb"], np.float32)  # [2, 8]

    for st, eattr_idx in (("ll", 1), ("lp", 0)):
        sp = params["egnn" if st == "ll" else "xatt"]
        ev = edge_emb[eattr_idx]  # [8]
        p[f"{st}_emb_w"] = f32(sp["emb"]["w"])              # [65, 64]
        p[f"{st}_emb_b"] = f32(sp["emb"]["b"])[:, None]     # [64, 1]
        if st == "lp":
            p["lp_kv_w"] = f32(sp["emb_kv"]["w"])           # [64, 64]
            p["lp_kv_b"] = f32(sp["emb_kv"]["b"])[:, None]
        p[f"{st}_out_w"] = f32(sp["out"]["w"])              # [64, 65]
        p[f"{st}_out_b"] = f32(sp["out"]["b"])[:, None]     # [65, 1]

        for bi, blk in enumerate(sp["blocks"]):
            units = [(f"g{gi}", g) for gi, g in enumerate(blk["gcls"])]
            units.append(("c", blk["coord"]))
            for uname, up in units:
                pre = f"{st}_b{bi}_{uname}"
                if uname == "c":
                    w1 = f32(up["w1"]); b1 = f32(up["b1"])
                    w2 = f32(up["w2"]); b2 = f32(up["b2"])
                else:
                    w1 = f32(up["e_w1"]); b1 = f32(up["e_b1"])
                    w2 = f32(up["e_w2"]); b2 = f32(up["e_b2"])
                Wa, Wb = w1[:64], w1[64:128]
                u, v = w1[128], w1[129]
                c0 = ev @ w1[130:138] + b1  # folded const [64]
                p[f"{pre}_wa"] = Wa                     # [64, 64] f32
                p[f"{pre}_wb"] = Wb
                for gp in range(2):
                    uvc = np.zeros((10, 128), np.float32)
                    uvc[2 * gp, :64] = u; uvc[2 * gp + 1, 64:] = u
                    uvc[4 + 2 * gp, :64] = v; uvc[5 + 2 * gp, 64:] = v
                    uvc[8, :64] = c0; uvc[8, 64:] = c0
                    p[f"{pre}_uvc{gp}"] = bf16(uvc)
                w2bd = np.zeros((128, 128), np.float32)
                w2bd[:64, :64] = w2; w2bd[64:, 64:] = w2
                p[f"{pre}_w2bd"] = bf16(w2bd)
                p[f"{pre}_b2bd"] = f32(np.concatenate([b2, b2]))[:, None]
                if uname == "c":
                    w3 = f32(up["w3"])  # [64, 1]
                    w3bd = np.zeros((128, 2), np.float32)
                    w3bd[:64, 0] = w3[:, 0]; w3bd[64:, 1] = w3[:, 0]
                    p[f"{pre}_w3bd"] = bf16(w3bd)
                else:
                    p[f"{pre}_c0bd"] = f32(np.concatenate([c0, c0]))[:, None]
                    nw1 = f32(up["n_w1"])   # [128, 64]
                    p[f"{pre}_nw1h"] = nw1[:64]
                    p[f"{pre}_nw1a"] = nw1[64:] / NORM_FACTOR
                    p[f"{pre}_nb1"] = f32(up["n_b1"])[:, None]
                    p[f"{pre}_nw2"] = f32(up["n_w2"])
                    p[f"{pre}_nb2"] = f32(up["n_b2"])[:, None]

    # rhs1 static images: [74, F] = indicator rows + zero rad/d0 rows + ones row
    def rhs1_img(nkv):
        img = np.zeros((74, 64 * nkv), np.float32)
        img[:64] = np.repeat(np.eye(64, dtype=np.float32), nkv, axis=1)
        img[72] = 1.0
        return bf16(img)
    p["ind_ll"] = rhs1_img(LIG_PER)
    p["ind_lp"] = rhs1_img(POC_PER)
    p["ones3"] = np.ones((3, 1), np.float32)
    p["ones_r"] = np.ones((1, 64), np.float32)
    return p


# ======================================================================
# bass builder
# ======================================================================
def _build(nc):
    import concourse.mybir as mybir
    import concourse.tile as tile
    from concourse.masks import make_identity

    F32 = mybir.dt.float32
    BF16 = mybir.dt.bfloat16
    AF = mybir.ActivationFunctionType
    OP = mybir.AluOpType
    AX = mybir.AxisListType

    D = {}  # dram tensors

    def din(name, shape, dt=F32):
        D[name] = nc.dram_tensor(name, list(shape), dt, kind="ExternalInput")
        return D[name]

    # ---- inputs
    din("hlT", [ATOM_NF, NLc]); din("hpT", [CTX_NF, NPc])
    din("tl", [1, NLc])
    din("xlT", [3, NLc]); din("xpT", [3, NPc])
    din("ind_ll", [74, 64 * LIG_PER], BF16)
    din("ind_lp", [74, 64 * POC_PER], BF16)
    din("ones3", [3, 1]); din("ones_r", [1, 64])
    for n, sh in [("ae_w1", [16, 32]), ("ae_b1", [32, 1]), ("ae_w2", [32, 64]),
                  ("ae_b2", [64, 1]), ("ce_w1", [21, 42]), ("ce_b1", [42, 1]),
                  ("ce_w2", [42, 64]), ("ce_b2", [64, 1]), ("ad_w1", [64, 32]),
                  ("ad_b1", [32, 1]), ("ad_w2", [32, 16]), ("ad_b2", [16, 1]),
                  ("lp_kv_w", [64, 64]), ("lp_kv_b", [64, 1])]:
        din(n, sh)
    for st in ("ll", "lp"):
        din(f"{st}_emb_w", [65, 64]); din(f"{st}_emb_b", [64, 1])
        din(f"{st}_out_w", [64, 65]); din(f"{st}_out_b", [65, 1])
        for bi in range(N_LAYERS):
            for uname in ("g0", "g1", "c"):
                pre = f"{st}_b{bi}_{uname}"
                din(f"{pre}_wa", [64, 64]); din(f"{pre}_wb", [64, 64])
                din(f"{pre}_uvc0", [10, 128], BF16); din(f"{pre}_uvc1", [10, 128], BF16)
                din(f"{pre}_w2bd", [128, 128], BF16)
                din(f"{pre}_b2bd", [128, 1])
                if uname == "c":
                    din(f"{pre}_w3bd", [128, 2], BF16)
                else:
                    din(f"{pre}_c0bd", [128, 1])
                    din(f"{pre}_nw1h", [64, 64]); din(f"{pre}_nw1a", [64, 64])
                    din(f"{pre}_nb1", [64, 1]); din(f"{pre}_nw2", [64, 64])
                    din(f"{pre}_nb2", [64, 1])
    out_d = nc.dram_tensor("out", [NLc, 3 + ATOM_NF], F32, kind="ExternalOutput")
    TAPS = {}
    if DEBUG_TAPS:
        for tn, tsh, tdt in ([("d_hlt", [65, NLc], F32), ("d_h0ll", [64, NLc], F32),
                             ("d_radll", [128, 64], F32), ("d_rhs1ll", [74, 512], BF16),
                             ("d_aggll", [128, 64], F32), ("d_zps", [128, 512], F32),
                             ("d_s1", [128, 512], BF16), ("d_hg1ll", [64, NLc], F32),
                             ("d_x1ll", [128, 3], F32), ("d_phill", [2, 4096], BF16),
                             ("d_d0ll", [74, 512], BF16),
                             ("d_phipm", [128, 64], BF16), ("d_th", [128, 64], F32),
                             ("d_rs", [128, 64], F32), ("d_sc", [128, 64], F32),
                             ("d_t1", [128, 3], F32), ("d_t2", [128, 3], F32),
                             ("d_stsb", [64, 128], F32),
                             ("d_hll_end", [64, NLc], F32), ("d_hlp_end", [64, NLc], F32),
                             ("d_xll_end", [128, 3], F32), ("d_xlp_end", [128, 3], F32),
                             ("d_xlp_b0", [128, 3], F32), ("d_agglp", [128, 64], F32)]
                            + [(f"d_hll_{bi}_{gi}", [64, NLc], F32)
                               for bi in range(4) for gi in range(2)]
                            + [(f"d_aggll_{bi}_{gi}", [128, 64], F32)
                               for bi in range(4) for gi in range(2)]
                            + [("d_zps01", [128, 512], F32), ("d_s101", [128, 512], BF16),
                               ("d_m01", [128, 512], F32), ("d_m00", [128, 512], F32),
                               ("d_at01", [64, 128], BF16), ("d_bt01", [64, 128], BF16)]):
            TAPS[tn] = nc.dram_tensor(tn, tsh, tdt, kind="ExternalOutput")

    with tile.TileContext(nc) as tc:
        with (
            tc.tile_pool(name="const", bufs=1) as cp,
            tc.tile_pool(name="work", bufs=3) as wp,
            tc.tile_pool(name="hpool", bufs=3) as hp,
            tc.tile_pool(name="encp", bufs=1) as ep,
            tc.tile_pool(name="phc", bufs=1) as phr,
            tc.tile_pool(name="xpool", bufs=3) as xp,
            tc.tile_pool(name="dram", bufs=2, space="DRAM") as dp,
            tc.tile_pool(name="pz", bufs=3, space="PSUM") as pz,
            tc.tile_pool(name="pm", bufs=3, space="PSUM") as pm,
            tc.tile_pool(name="ps", bufs=2, space="PSUM") as ps,
        ):
            S = {}  # persistent sbuf tiles

            def tap(name, ap, psum=False):
                if not DEBUG_TAPS or name not in TAPS:
                    return
                if psum:
                    shp = list(TAPS[name].shape)
                    tt = wp.tile(shp, TAPS[name].dtype, tag="tapp", name="tt")
                    nc.vector.tensor_copy(tt[:], ap)
                    ap = tt[:]
                nc.sync.dma_start(TAPS[name][:], ap)

            def ctile(name, shape, dt=F32):
                t = cp.tile(list(shape), dt, tag=name)
                S[name] = t
                return t

            def load(name, dt=None):
                d = D[name]
                t = ctile(name, d.shape, dt or d.dtype)
                nc.sync.dma_start(t[:], d[:])
                return t

            # ---- constants / weights into SBUF
            ident = ctile("ident", [128, 128])
            make_identity(nc, ident[:])
            cb_ln = ctile("cb_ln", [1, 1])
            nc.vector.memset(cb_ln[:], 1e-5)
            cb_rad = ctile("cb_rad", [128, 1])
            nc.vector.memset(cb_rad[:], 1e-8 * KSC * KSC)
            identb = ctile("identb", [128, 128], BF16)
            nc.vector.tensor_copy(identb[:], ident[:])

            for n in D:
                if n not in ("ind_ll", "ind_lp"):
                    load(n)

            # ================= encoders (feature-major) =================
            def enc_mlp(w1, b1, w2, b2, src, n_nodes, f_in, f_mid):
                e1p = ps.tile([f_mid, n_nodes], F32, tag="psmall")
                nc.tensor.matmul(e1p[:], S[w1][:], src[:], start=True, stop=True)
                s1 = ep.tile([f_mid, n_nodes], F32, tag="enc_s1")
                nc.scalar.activation(s1[:], e1p[:], AF.Silu, bias=S[b1][:])
                e2p = ps.tile([64, n_nodes], F32, tag="psmall")
                nc.tensor.matmul(e2p[:], S[w2][:], s1[:], start=True, stop=True)
                # clip +-50
                emb = ep.tile([64, n_nodes], F32, tag="enc_emb")
                nc.scalar.activation(emb[:], e2p[:], AF.Identity, bias=S[b2][:])
                nc.vector.tensor_scalar(emb[:], emb[:], 50.0, -50.0, OP.min, OP.max)
                # layernorm over the 64 features (partition dim) via matmul
                ones_col = cp.tile([64, 2], F32, tag="ln_ones")
                nc.vector.memset(ones_col[:, 0:1], 1.0 / 64)
                nc.vector.memset(ones_col[:, 1:2], 0.0)
                sq = ep.tile([64, n_nodes], F32, tag="enc_sq")
                nc.vector.tensor_tensor(sq[:], emb[:], emb[:], OP.mult)
                mvp0 = ps.tile([1, n_nodes], F32, tag="psmall")
                nc.tensor.matmul(mvp0[:], ones_col[:, 0:1], emb[:], start=True, stop=True)
                mvp1 = ps.tile([1, n_nodes], F32, tag="psmall")
                nc.tensor.matmul(mvp1[:], ones_col[:, 0:1], sq[:], start=True, stop=True)
                mva = ep.tile([1, n_nodes], F32, tag="enc_mva")
                nc.vector.tensor_copy(mva[:], mvp0[:])
                mvb = ep.tile([1, n_nodes], F32, tag="enc_mvb")
                nc.vector.tensor_copy(mvb[:], mvp1[:])
                var = ep.tile([1, n_nodes], F32, tag="enc_var")
                nc.vector.tensor_tensor(var[:], mva[:], mva[:], OP.mult)
                nc.vector.tensor_tensor(var[:], mvb[:], var[:], OP.subtract)
                rstd = ep.tile([1, n_nodes], F32, tag="enc_rstd")
                nc.scalar.activation(rstd[:], var[:], AF.Sqrt, bias=S["cb_ln"][:])
                nc.vector.reciprocal(rstd[:], rstd[:])
                # broadcast mean/rstd across 64 partitions (gpsimd)
                mb = ep.tile([64, n_nodes], F32, tag="enc_mb")
                rb = ep.tile([64, n_nodes], F32, tag="enc_rb")
                nc.gpsimd.partition_broadcast(mb[:], mva[:])
                nc.gpsimd.partition_broadcast(rb[:], rstd[:])
                nc.vector.tensor_tensor(emb[:], emb[:], mb[:], OP.subtract)
                nc.vector.tensor_tensor(emb[:], emb[:], rb[:], OP.mult)
                return emb

            hl_emb = enc_mlp("ae_w1", "ae_b1", "ae_w2", "ae_b2", S["hlT"], NLc, 16, 32)
            hp_emb = enc_mlp("ce_w1", "ce_b1", "ce_w2", "ce_b2", S["hpT"], NPc, 21, 42)

            # h_l_t = [hl_emb ; t]  -> [65, 256]
            hlt = ctile("hlt", [65, NLc])
            nc.vector.tensor_copy(hlt[0:64, :], hl_emb[:])
            nc.vector.tensor_copy(hlt[64:65, :], S["tl"][:])
            tap("d_hlt", hlt[:])

            # ================= geometry state =================
            # x_pm per graph-pair [128, 3]; x_fm [6, n-per-graph]
            def transpose_to_pm(src_a, src_b, gp):
                """two [3, 64] fm tiles -> packed pair-major [128, 3]"""
                t = xp.tile([128, 3], F32, tag=f"xpm_t_{gp}")
                pp = ps.tile([64, 3], F32, tag="psmall")
                nc.tensor.transpose(pp[:], src_a[:], ident[:3, :3])
                nc.vector.tensor_copy(t[0:64, :], pp[:])
                pp2 = ps.tile([64, 3], F32, tag="psmall")
                nc.tensor.transpose(pp2[:], src_b[:], ident[:3, :3])
                nc.vector.tensor_copy(t[64:128, :], pp2[:])
                return t

            # lig x_fm per gp: [6, 64]
            x0l_fm, x0l_pm = [], []
            for gp in range(2):
                xfa = ctile(f"x0l_fma_{gp}", [3, 64])
                xfb = ctile(f"x0l_fmb_{gp}", [3, 64])
                nc.vector.tensor_copy(xfa[:], S["xlT"][:, (2 * gp) * 64:(2 * gp + 1) * 64])
                nc.vector.tensor_copy(xfb[:], S["xlT"][:, (2 * gp + 1) * 64:(2 * gp + 2) * 64])
                x0l_fm.append((xfa, xfb))
                x0l_pm.append(transpose_to_pm(xfa, xfb, f"x0_{gp}"))
            # pocket x (static): fm [6, 128] per gp; pm per graph [128, 3] -> per gp two [128,3]? need xk_pm per graph [nkv,3]
            xp_fm, xp_pm = [], []
            for gp in range(2):
                xfa = ctile(f"xp_fma_{gp}", [3, POC_PER])
                xfb = ctile(f"xp_fmb_{gp}", [3, POC_PER])
                nc.vector.tensor_copy(xfa[:], S["xpT"][:, (2 * gp) * 128:(2 * gp + 1) * 128])
                nc.vector.tensor_copy(xfb[:], S["xpT"][:, (2 * gp + 1) * 128:(2 * gp + 2) * 128])
                xp_fm.append((xfa, xfb))
                pms = []
                for g, xf in enumerate((xfa, xfb)):
                    t = ctile(f"xp_pm_{gp}_{g}", [128, 3])
                    pp = ps.tile([128, 3], F32, tag="psmall")
                    nc.tensor.transpose(pp[:], xf[:], ident[:3, :3])
                    nc.vector.tensor_copy(t[:], pp[:])
                    pms.append(t)
                xp_pm.append(pms)

            # ================= message-passing stacks =================
            results = {}

            def run_stack(st):
                cross = st == "lp"
                NKV = POC_PER if cross else LIG_PER
                Fg = 64 * NKV                   # pairs per graph
                ICH = 512 // NKV                # i's per 512-chunk
                NCH = Fg // 512                 # chunks per graph-pair
                ind_name = "ind_lp" if cross else "ind_ll"

                # h0 = h_l_t @ emb_w + emb_b   [64, 256]
                h0p = ps.tile([64, NLc], F32, tag="psmall")
                nc.tensor.matmul(h0p[:], S[f"{st}_emb_w"][:], hlt[:], start=True, stop=True)
                h = hp.tile([64, NLc], F32, tag=f"h_{st}")
                nc.scalar.activation(h[:], h0p[:], AF.Identity, bias=S[f"{st}_emb_b"][:])
                if st == "ll":
                    tap("d_h0ll", h[:])

                if cross:
                    hkp = ps.tile([64, NPc], F32, tag="psmall")
                    nc.tensor.matmul(hkp[:], S["lp_kv_w"][:], hp_emb[:], start=True, stop=True)
                    hk = ctile("hk_lp", [64, NPc])
                    nc.scalar.activation(hk[:], hkp[:], AF.Identity, bias=S["lp_kv_b"][:])

                # rhs1 (shared): [74, Fg] bf16; rows: 0-63 indicator,
                # 64-67 rad gp0/gp1, 68-71 d0 gp0/gp1, 72 ones, 73 zero
                rhs1 = ctile(f"rhs1_{st}", [74, Fg], BF16)
                nc.sync.dma_start(rhs1[:], D[ind_name][:])

                # gram machinery per gp (persistent tiles)


                def gram_rad(gp, xq_fm, xq_pm, xk_fm, rows_lo, f32_out):
                    """rad for graph-pair gp -> (rad_pm f32 [128, NKV]).
                    Also writes bf16 flatten into rhs1 rows [rows_lo, rows_lo+2)."""
                    xqa, xqb = xq_fm
                    xka, xkb = xk_fm
                    gpp = ps.tile([128, NKV], F32, tag="psmall")
                    for half, (xq_h, xk_h) in enumerate(((xqa, xka), (xqb, xkb))):
                        glh = wp.tile([3, 64], F32, tag="glh")
                        nc.vector.tensor_scalar_mul(glh[:], xq_h[:], -2.0)
                        sqk = wp.tile([3, NKV], F32, tag=f"sqk_{st}")
                        nc.vector.tensor_tensor(sqk[:], xk_h[:], xk_h[:], OP.mult)
                        q2p = ps.tile([1, NKV], F32, tag="psmall")
                        nc.tensor.matmul(q2p[:], S["ones3"][:], sqk[:], start=True, stop=True)
                        q2 = wp.tile([1, NKV], F32, tag="q2")
                        nc.vector.tensor_copy(q2[:], q2p[:])
                        lo_p = 64 * half
                        nc.tensor.matmul(gpp[lo_p:lo_p + 64, :], glh[:], xk_h[:],
                                         start=True, stop=False)
                        nc.tensor.matmul(gpp[lo_p:lo_p + 64, :], S["ones_r"][:], q2[:],
                                         start=False, stop=True)
                    # |x_i|^2 per-partition bias
                    xsq = wp.tile([128, 3], F32, tag="xsq")
                    nc.vector.tensor_tensor(xsq[:], xq_pm[:], xq_pm[:], OP.mult)
                    xq2 = wp.tile([128, 1], F32, tag="xq2")
                    nc.vector.tensor_reduce(xq2[:], xsq[:], AX.X, OP.add)
                    radf = None
                    if f32_out:
                        radf = wp.tile([128, NKV], F32, tag=f"radf_{st}", name="radf")
                    radb = wp.tile([128, NKV], BF16, tag=f"radb_{st}")
                    nc.scalar.activation(radb[:], gpp[:], AF.Identity, bias=xq2[:])
                    if f32_out:
                        nc.scalar.activation(radf[:], gpp[:], AF.Identity, bias=xq2[:])
                        nc.vector.tensor_scalar_max(radf[:], radf[:], 0.0)
                    lo = rows_lo + 2 * gp
                    db = dp.tile([128, NKV], BF16, tag=f"radd_{st}")
                    nc.sync.dma_start(db[:], radb[:])
                    nc.sync.dma_start(
                        rhs1[lo:lo + 2, :],
                        db[:].rearrange("(two i) j -> two (i j)", two=2))
                    return radf

                def make_AT(hsrc, w_name, tagpre, gp, n_m):
                    """A_T [n_m, 128] psum from feature-major h [64, ...]."""
                    atp = ps.tile([n_m, 128], F32, tag="psmall")
                    c0 = (2 * gp) * n_m
                    nc.tensor.matmul(atp[:, 0:64], hsrc[:, c0:c0 + n_m],
                                     S[w_name][:], start=True, stop=True)
                    nc.tensor.matmul(atp[:, 64:128], hsrc[:, c0 + n_m:c0 + 2 * n_m],
                                     S[w_name][:], start=True, stop=True)
                    return atp

                # initial geometry per stack
                xq_fm = [x0l_fm[0], x0l_fm[1]]
                xq_pm = [x0l_pm[0], x0l_pm[1]]
                if cross:
                    xk_fm = xp_fm
                else:
                    xk_fm = xq_fm

                # d0 rows (once per stack)
                for gp in range(2):
                    gram_rad(gp, xq_fm[gp], xq_pm[gp], xk_fm[gp], 68, False)
                if st == "ll":
                    tap("d_d0ll", rhs1[:, 0:512])

                bt_cache = {}

                for bi in range(N_LAYERS):
                    # ---- block prologue: rad + coord scalars per gp
                    rs_all = []
                    for gp in range(2):
                        radf = gram_rad(gp, xq_fm[gp], xq_pm[gp], xk_fm[gp], 64, True)
                        if st == "ll" and bi == 0 and gp == 0:
                            tap("d_radll", radf[:])
                            tap("d_rhs1ll", rhs1[:, 0:512])
                        # rs = (CR/100) / (sqrt(rad+1e-8) + 1), sqrt via DVE
                        # newton-rsqrt (keeps ACT on the silu table set)
                        I32 = mybir.dt.int32
                        re_ = wp.tile([128, NKV], F32, tag=f"den_{st}", name="re_")
                        nc.vector.tensor_scalar_add(re_[:], radf[:], 1e-8)
                        yv = wp.tile([128, NKV], F32, tag=f"yv_{st}", name="yv")
                        nc.vector.tensor_scalar(yv[:].bitcast(I32), re_[:].bitcast(I32),
                                                1, None, OP.arith_shift_right)
                        nc.vector.tensor_scalar(yv[:].bitcast(I32), yv[:].bitcast(I32),
                                                -1, 0x5F3759DF, OP.mult, OP.add)
                        t_ = wp.tile([128, NKV], F32, tag=f"tn_{st}", name="t_")
                        for _ in range(2):
                            nc.vector.tensor_tensor(t_[:], yv[:], yv[:], OP.mult)
                            nc.vector.tensor_tensor(t_[:], t_[:], re_[:], OP.mult)
                            nc.vector.tensor_scalar(t_[:], t_[:], -0.5, 1.5,
                                                    OP.mult, OP.add)
                            nc.vector.tensor_tensor(yv[:], yv[:], t_[:], OP.mult)
                        den = wp.tile([128, NKV], F32, tag=f"dn_{st}", name="den")
                        nc.vector.tensor_tensor(den[:], re_[:], yv[:], OP.mult)
                        nc.vector.tensor_scalar(den[:], den[:], KSC, KSC,
                                                OP.mult, OP.add)
                        nc.vector.reciprocal(den[:], den[:])
                        rs_all.append(den)

                    def pair_mlp(pre, gp, at_sb, bt_sb, want_m):
                        """Run chunked pair MLP. Returns agg [128, 64] (if want_m)
                        else phi_row [2, Fg]."""
                        # lhs1 = [A_T ; uvc] bf16
                        lhs1 = wp.tile([74, 128], BF16, tag="lhs1")
                        nc.vector.tensor_copy(lhs1[0:64, :], at_sb[:])
                        nc.vector.tensor_copy(lhs1[64:74, :], S[f"{pre}_uvc{gp}"][:])
                        agg = None
                        phi_row = None
                        if want_m:
                            agg = wp.tile([128, 64], F32, tag="agg", name="agg")
                        else:
                            phi_row = phr.tile([2, Fg], BF16, tag=f"phi_row_{st}",
                                              name="phi_row")
                        for ci in range(NCH):
                            zps = pz.tile([128, 512], F32, tag="zchunk")
                            nc.tensor.matmul(zps[:], lhs1[:],
                                             rhs1[:, ci * 512:(ci + 1) * 512],
                                             start=True, stop=False)
                            rhsb = identb[:NKV, None, :NKV].to_broadcast(
                                (NKV, ICH, NKV))
                            nc.tensor.matmul(zps[:].rearrange("p (i j) -> p i j", i=ICH),
                                             bt_sb[:], rhsb, start=False, stop=True)
                            if (DEBUG_TAPS and st == "ll" and ci == 0 and gp == 0
                                    and pre.endswith("b0_g0")):
                                tap("d_zps", zps[:], psum=True)
                            s1 = wp.tile([128, 512], BF16, tag="s1")
                            nc.scalar.activation(s1[:], zps[:], AF.Silu)
                            if (DEBUG_TAPS and st == "ll" and ci == 0 and gp == 0
                                    and pre.endswith("b0_g0")):
                                tap("d_s1", s1[:])
                            if (DEBUG_TAPS and st == "ll" and ci == 0 and gp == 0
                                    and pre.endswith("b0_g1")):
                                tap("d_s101", s1[:])
                            mps = pm.tile([128, 512], F32, tag="mchunk")
                            nc.tensor.matmul(mps[:], S[f"{pre}_w2bd"][:], s1[:],
                                             start=True, stop=True)
                            if want_m:
                                mt = wp.tile([128, 512], BF16, tag="mt")
                                nc.scalar.activation(mt[:], mps[:], AF.Silu,
                                                     bias=S[f"{pre}_b2bd"][:])
                                if (DEBUG_TAPS and st == "ll" and ci == 0 and gp == 0
                                        and bi == 0):
                                    tap(f"d_m0{gi}", mt[:])
                                nc.vector.tensor_reduce(
                                    agg[:, ci * ICH:(ci + 1) * ICH],
                                    mt[:].rearrange("p (i j) -> p i j", i=ICH),
                                    AX.X, OP.add)
                            else:
                                s2 = wp.tile([128, 512], BF16, tag="s2")
                                nc.scalar.activation(s2[:], mps[:], AF.Silu,
                                                     bias=S[f"{pre}_b2bd"][:])
                                php = ps.tile([2, 512], F32, tag="psmall")
                                nc.tensor.matmul(php[:], S[f"{pre}_w3bd"][:], s2[:],
                                                 start=True, stop=True)
                                nc.vector.tensor_copy(
                                    phi_row[:, ci * 512:(ci + 1) * 512], php[:])
                        return agg if want_m else phi_row

                    # ---- gcl units
                    for gi in range(INV_SUB):
                        pre = f"{st}_b{bi}_g{gi}"
                        aggs = []
                        for gp in range(2):
                            atp = make_AT(h, f"{pre}_wa", "at", gp, 64)
                            at_sb = wp.tile([64, 128], BF16, tag="at_sb")
                            nc.vector.tensor_copy(at_sb[:], atp[:])
                            if st == "ll" and bi == 0 and gi == 1 and gp == 0:
                                tap("d_at01", at_sb[:])
                            if cross:
                                key = (bi, gi, gp)
                                if key not in bt_cache:
                                    btp = make_AT(hk, f"{pre}_wb", "bt", gp, POC_PER)
                                    bt_sb = ctile(f"btc_{st}_{bi}_{gi}_{gp}",
                                                  [POC_PER, 128], BF16)
                                    nc.vector.tensor_copy(bt_sb[:], btp[:])
                                    bt_cache[key] = bt_sb
                                bt_sb = bt_cache[key]
                            else:
                                btp = make_AT(h, f"{pre}_wb", "bt", gp, 64)
                                bt_sb = wp.tile([64, 128], BF16, tag="bt_sb")
                                nc.vector.tensor_copy(bt_sb[:], btp[:])
                                if st == "ll" and bi == 0 and gi == 1 and gp == 0:
                                    tap("d_bt01", bt_sb[:])
                            agg = pair_mlp(pre, gp, at_sb, bt_sb, True)
                            if not cross:
                                # diagonal correction: subtract m_ii
                                zd = wp.tile([64, 128], BF16, tag="zd")
                                nc.vector.tensor_tensor(zd[:], at_sb[:], bt_sb[:], OP.add)
                                zdp = ps.tile([128, 64], BF16, tag="psmall")
                                nc.tensor.transpose(zdp[:], zd[:], identb[:64, :64])
                                s1d = wp.tile([128, 64], BF16, tag="s1d")
                                nc.scalar.activation(s1d[:], zdp[:], AF.Silu,
                                                     bias=S[f"{pre}_c0bd"][:])
                                mdp = ps.tile([128, 64], F32, tag="psmall")
                                nc.tensor.matmul(mdp[:], S[f"{pre}_w2bd"][:], s1d[:],
                                                 start=True, stop=True)
                                md = wp.tile([128, 64], F32, tag="md")
                                nc.scalar.activation(md[:], mdp[:], AF.Silu,
                                                     bias=S[f"{pre}_b2bd"][:])
                                nc.vector.tensor_tensor(agg[:], agg[:], md[:], OP.subtract)
                            if st == "ll" and bi == 0 and gi == 0 and gp == 0:
                                tap("d_aggll", agg[:])
                            if st == "ll" and gp == 0:
                                tap(f"d_aggll_{bi}_{gi}", agg[:])
                            if st == "lp" and bi == 0 and gi == 0 and gp == 0:
                                tap("d_agglp", agg[:])
                            aggs.append(agg)
                        # node update (all 4 graphs)
                        agg_all = wp.tile([64, NLc], F32, tag="agg_all")
                        for gp in range(2):
                            nc.vector.tensor_copy(
                                agg_all[:, (2 * gp) * 64:(2 * gp + 1) * 64],
                                aggs[gp][0:64, :])
                            nc.vector.tensor_copy(
                                agg_all[:, (2 * gp + 1) * 64:(2 * gp + 2) * 64],
                                aggs[gp][64:128, :])
                        np1 = ps.tile([64, NLc], F32, tag="psmall")
                        nc.tensor.matmul(np1[:], S[f"{pre}_nw1h"][:], h[:],
                                         start=True, stop=False)
                        nc.tensor.matmul(np1[:], S[f"{pre}_nw1a"][:], agg_all[:],
                                         start=False, stop=True)
                        sn = wp.tile([64, NLc], F32, tag="sn")
                        nc.scalar.activation(sn[:], np1[:], AF.Silu, bias=S[f"{pre}_nb1"][:])
                        np2 = ps.tile([64, NLc], F32, tag="psmall")
                        nc.tensor.matmul(np2[:], S[f"{pre}_nw2"][:], sn[:],
                                         start=True, stop=True)
                        h_new = hp.tile([64, NLc], F32, tag=f"h_{st}")
                        nc.vector.scalar_tensor_tensor(h_new[:], np2[:], S[f"{pre}_nb2"][:],
                                                       h[:], OP.add, OP.add)
                        h = h_new
                        if st == "ll" and bi == 0 and gi == 0:
                            tap("d_hg1ll", h[:])
                        if st == "ll":
                            tap(f"d_hll_{bi}_{gi}", h[:])

                    # ---- coord unit
                    pre = f"{st}_b{bi}_c"
                    new_xq_fm, new_xq_pm = [], []
                    for gp in range(2):
                        atp = make_AT(h, f"{pre}_wa", "at", gp, 64)
                        at_sb = wp.tile([64, 128], BF16, tag="at_sb")
                        nc.vector.tensor_copy(at_sb[:], atp[:])
                        if cross:
                            key = ("c", bi, gp)
                            if key not in bt_cache:
                                btp = make_AT(hk, f"{pre}_wb", "bt", gp, POC_PER)
                                bt_sb = ctile(f"btc_{st}_c{bi}_{gp}", [POC_PER, 128], BF16)
                                nc.vector.tensor_copy(bt_sb[:], btp[:])
                                bt_cache[key] = bt_sb
                            bt_sb = bt_cache[key]
                        else:
                            btp = make_AT(h, f"{pre}_wb", "bt", gp, 64)
                            bt_sb = wp.tile([64, 128], BF16, tag="bt_sb")
                            nc.vector.tensor_copy(bt_sb[:], btp[:])
                        phi_row = pair_mlp(pre, gp, at_sb, bt_sb, False)
                        # phi -> pair-major
                        phi_pm = wp.tile([128, NKV], BF16, tag=f"phi_pm_{st}")
                        db2 = dp.tile([2, Fg], BF16, tag=f"phid_{st}")
                        nc.sync.dma_start(db2[:], phi_row[:])
                        nc.sync.dma_start(
                            phi_pm[:],
                            db2[:].rearrange("two (i j) -> (two i) j", i=64))
                        if st == "ll" and bi == 0 and gp == 0:
                            tap("d_phill", phi_row[:])
                        th = wp.tile([128, NKV], F32, tag=f"th_{st}")
                        nc.scalar.activation(th[:], phi_pm[:], AF.Tanh)
                        sc = wp.tile([128, NKV], F32, tag=f"sc_{st}")
                        nc.vector.tensor_tensor(sc[:], th[:], rs_all[gp][:], OP.mult)
                        if st == "ll" and bi == 0 and gp == 0:
                            tap("d_phipm", phi_pm[:])
                            tap("d_th", th[:])
                            tap("d_rs", rs_all[gp][:])
                            tap("d_sc", sc[:])
                        # term1 = x_pm * rowsum(sc)
                        srow = wp.tile([128, 1], F32, tag="srow")
                        nc.vector.tensor_reduce(srow[:], sc[:], AX.X, OP.add)
                        t1 = wp.tile([128, 3], F32, tag="t1")
                        nc.vector.tensor_tensor(
                            t1[:], xq_pm[gp][:],
                            srow[:, 0:1].to_broadcast((128, 3)), OP.mult)
                        # term2 = sum_j sc[i,j] * xk_pm[j,:]
                        stp = ps.tile([NKV, 128], F32, tag="psmall")
                        nc.tensor.transpose(stp[:], sc[:], ident[:])
                        st_sb = wp.tile([NKV, 128], F32, tag=f"st_sb_{st}")
                        nc.vector.tensor_copy(st_sb[:], stp[:])
                        t2p = ps.tile([128, 3], F32, tag="psmall")
                        if cross:
                            xk_pm_A, xk_pm_B = xp_pm[gp][0], xp_pm[gp][1]
                        else:
                            xk_pm_A, xk_pm_B = None, None  # self: from xq_pm halves
                        if cross:
                            nc.tensor.matmul(t2p[0:64, :], st_sb[:, 0:64], xk_pm_A[:],
                                             start=True, stop=True)
                            nc.tensor.matmul(t2p[64:128, :], st_sb[:, 64:128], xk_pm_B[:],
                                             start=True, stop=True,
                                             tile_position=(0, 64))
                        else:
                            xkA = wp.tile([64, 3], F32, tag="xkA")
                            nc.vector.tensor_copy(xkA[:], xq_pm[gp][0:64, :])
                            xkB = wp.tile([64, 3], F32, tag="xkB")
                            nc.vector.tensor_copy(xkB[:], xq_pm[gp][64:128, :])
                            nc.tensor.matmul(t2p[0:64, :], st_sb[:, 0:64], xkA[:],
                                             start=True, stop=True)
                            nc.tensor.matmul(t2p[64:128, :], st_sb[:, 64:128], xkB[:],
                                             start=True, stop=True,
                                             tile_position=(0, 64))
                        if st == "ll" and bi == 0 and gp == 0:
                            tap("d_t1", t1[:])
                            tap("d_t2", t2p[:], psum=True)
                            tap("d_stsb", st_sb[:])
                        x_new = xp.tile([128, 3], F32, tag=f"xpm_{st}_{gp}")
                        nc.vector.tensor_tensor(x_new[:], xq_pm[gp][:], t1[:], OP.add)
                        nc.vector.tensor_tensor(x_new[:], x_new[:], t2p[:], OP.subtract)
                        new_xq_pm.append(x_new)
                        if st == "ll" and bi == 0 and gp == 0:
                            tap("d_x1ll", x_new[:])
                        if st == "lp" and bi == 0 and gp == 0:
                            tap("d_xlp_b0", x_new[:])
                        # back to feature-major (two [3, 64] tiles)
                        xfa = xp.tile([3, 64], F32, tag=f"xfma_{st}_{gp}")
                        fp1 = ps.tile([3, 64], F32, tag="psmall")
                        nc.tensor.transpose(fp1[:], x_new[0:64, :], ident[:64, :64])
                        nc.vector.tensor_copy(xfa[:], fp1[:])
                        xfb = xp.tile([3, 64], F32, tag=f"xfmb_{st}_{gp}")
                        fp2 = ps.tile([3, 64], F32, tag="psmall")
                        nc.tensor.transpose(fp2[:], x_new[64:128, :], ident[64:128, 64:128])
                        nc.vector.tensor_copy(xfb[:], fp2[:])
                        new_xq_fm.append((xfa, xfb))
                    xq_fm, xq_pm = new_xq_fm, new_xq_pm
                    if not cross:
                        xk_fm = xq_fm

                tap(f"d_h{st}_end", h[:])
                tap(f"d_x{st}_end", xq_pm[0][:])
                # stack epilogue: h65 = h @ out_w + out_b
                h65p = ps.tile([65, NLc], F32, tag="psmall")
                nc.tensor.matmul(h65p[:], S[f"{st}_out_w"][:], h[:], start=True, stop=True)
                h65 = ctile(f"h65_{st}", [65, NLc])
                nc.scalar.activation(h65[:], h65p[:], AF.Identity, bias=S[f"{st}_out_b"][:])
                results[st] = (h65, xq_pm)

            run_stack("ll")
            run_stack("lp")

            # ================= final combine =================
            h65_ll, x_ll_pm = results["ll"]
            h65_lp, x_lp_pm = results["lp"]
            hfin = cp.tile([64, NLc], F32, tag="hfin")
            tmp = wp.tile([64, NLc], F32, tag="cmb_tmp")
            nc.vector.tensor_scalar_mul(tmp[:], h65_lp[0:64, :], 0.4)
            nc.vector.scalar_tensor_tensor(hfin[:], h65_ll[0:64, :], 0.6, tmp[:],
                                           OP.mult, OP.add)
            # feats = atom_dec MLP
            f1p = ps.tile([32, NLc], F32, tag="psmall")
            nc.tensor.matmul(f1p[:], S["ad_w1"][:], hfin[:], start=True, stop=True)
            f1 = wp.tile([32, NLc], F32, tag="f1")
            nc.scalar.activation(f1[:], f1p[:], AF.Silu, bias=S["ad_b1"][:])
            f2p = ps.tile([16, NLc], F32, tag="psmall")
            nc.tensor.matmul(f2p[:], S["ad_w2"][:], f1[:], start=True, stop=True)
            feats = cp.tile([16, NLc], F32, tag="feats")
            nc.scalar.activation(feats[:], f2p[:], AF.Identity, bias=S["ad_b2"][:])
            velf = cp.tile([3, NLc], F32, tag="velf")
            # vel rows: per gp compute vel_pm then transpose into outf rows 0-2
            for gp in range(2):
                d1 = wp.tile([128, 3], F32, tag="veld1")
                nc.vector.tensor_tensor(d1[:], x_ll_pm[gp][:], x0l_pm[gp][:], OP.subtract)
                d2 = wp.tile([128, 3], F32, tag="veld2")
                nc.vector.tensor_tensor(d2[:], x_lp_pm[gp][:], x0l_pm[gp][:], OP.subtract)
                nc.vector.tensor_scalar_mul(d2[:], d2[:], 0.4)
                vel = wp.tile([128, 3], F32, tag="vel")
                nc.vector.scalar_tensor_tensor(vel[:], d1[:], 0.6, d2[:], OP.mult, OP.add)
                vp1 = ps.tile([3, 64], F32, tag="psmall")
                nc.tensor.transpose(vp1[:], vel[0:64, :], ident[:64, :64])
                nc.vector.tensor_copy(velf[:, (2 * gp) * 64:(2 * gp + 1) * 64], vp1[:])
                vp2 = ps.tile([3, 64], F32, tag="psmall")
                nc.tensor.transpose(vp2[:], vel[64:128, :], ident[64:128, 64:128])
                nc.vector.tensor_copy(velf[:, (2 * gp + 1) * 64:(2 * gp + 2) * 64], vp2[:])
            # transpose [vel | feats] -> out [256, 19] (free-dim concat in ot)
            for half in range(2):
                ot = wp.tile([128, 19], F32, tag="ot")
                opv = ps.tile([128, 3], F32, tag="psmall")
                nc.tensor.transpose(opv[:], velf[:, half * 128:(half + 1) * 128],
                                    ident[:3, :3])
                nc.vector.tensor_copy(ot[:, 0:3], opv[:])
                opf = ps.tile([128, 16], F32, tag="psmall")
                nc.tensor.transpose(opf[:], feats[:, half * 128:(half + 1) * 128],
                                    ident[:16, :16])
                nc.vector.tensor_copy(ot[:, 3:19], opf[:])
                nc.sync.dma_start(out_d[half * 128:(half + 1) * 128, :], ot[:])

    nc.compile()
    return nc


# ======================================================================
# public entry
# ======================================================================
def kernel(xh_lig, xh_context, t, mask_lig, mask_context, edges_ll, edges_lp,
           params):
    import jax

    xh_lig = np.asarray(xh_lig, np.float32)
    xh_context = np.asarray(xh_context, np.float32)
    t_np = np.asarray(t, np.float32)
    ml = np.asarray(mask_lig)

    # deterministic reference noise (key 42), computed on CPU
    cpu = jax.local_devices(backend="cpu")[0]
    with jax.default_device(cpu):
        k1, k2 = jax.random.split(jax.random.key(42))
        nl = 1e-3 * np.asarray(jax.random.normal(k1, (xh_lig.shape[0], NDIMS),
                                                 jax.numpy.float32))
        npk = 1e-3 * np.asarray(jax.random.normal(k2, (xh_context.shape[0], NDIMS),
                                                  jax.numpy.float32))
    x_l = xh_lig[:, :NDIMS] + nl
    x_p = xh_context[:, :NDIMS] + npk
    h_l = xh_lig[:, NDIMS:]
    h_p = xh_context[:, NDIMS:]
    t_lig = t_np[ml, 0]  # [2048]

    if "nc" not in _CACHE:
        import concourse.bacc as bacc
        nc = bacc.Bacc(None, target_bir_lowering=False)
        _CACHE["nc"] = _build(nc)
    nc = _CACHE["nc"]

    if "pp" not in _CACHE:
        _CACHE["pp"] = _prep_params(params)
    pp = _CACHE["pp"]

    in_maps = []
    for c in range(N_CORES):
        ls, le = c * NLc, (c + 1) * NLc
        ps_, pe = c * NPc, (c + 1) * NPc
        m = dict(pp)
        m["hlT"] = np.ascontiguousarray(h_l[ls:le].T)
        m["hpT"] = np.ascontiguousarray(h_p[ps_:pe].T)
        m["tl"] = np.ascontiguousarray(t_lig[ls:le])[None, :]
        m["xlT"] = np.ascontiguousarray(x_l[ls:le].T)
        m["xpT"] = np.ascontiguousarray(x_p[ps_:pe].T)
        in_maps.append(m)

    from concourse.bass_utils import run_bass_kernel_spmd
    res = run_bass_kernel_spmd(nc, in_maps, core_ids=list(range(N_CORES)),
                               trace=bool(os.environ.get("KTRACE")))
    _CACHE["last_results"] = res

    lig_update = np.concatenate([res.results[c]["out"] for c in range(N_CORES)],
                                axis=0).astype(np.float32)
    pocket_update = np.zeros_like(xh_context)
    return lig_update, pocket_update
